# revision 6
# baseline (speedup 1.0000x reference)
"""Luong 'general' attention scores + softmax on 8 Trainium2 NeuronCores.

Reference computes:
    energy = einsum('sbh,kh->sbk', enc, W) + b          # [S,B,H]
    scores = einsum('bh,sbh->bs', hidden[0], energy)    # [B,S]
    attn   = softmax(scores, axis=1)[:, None, :]        # [B,1,S]

Algebra: scores[b,s] = hidden[b] . (W @ enc[s,b]) + hidden[b] . bias.
The bias term is constant over s, so it cancels in the softmax.  With
q = hidden @ W  (tiny matmul), scores[b,s] = q[b] . enc[s,b].  The kernel
streams enc exactly once as fp16 (empirical rel err ~5.5e-3 against the
2e-2 gate, half the fp32 bytes).

Measurement note (2026-08-10): timing through run_bass_kernel_spmd
re-traces the program and re-uploads all inputs every call; an earlier
session's "DMA caps at 120-130 GB/s/core" and 126 us/iter were artifacts
of that.  With a persistent jitted callable + device-resident inputs +
pipelined dispatch (runner.py/bench.py), DMA-only probes sustain
~350-480 GB/s/core on one HWDGE ring and ~560-900 GB/s/core across both
HWDGE rings (sync+scalar), all 8 cores streaming.  The previous all-DVE
kernel then measured 37-52 us and was DVE-bound: every s-column's
multiply+reduce ran on DVE (64 cols x [128,1024] fp16 at ~245 G elem/s
~= 34 us).  GPSIMD Pool tensor_tensor offload is a hardware-verified
loss (94 us).  PE, however, is idle after the tiny q matmuls.

This version is a DVE/PE hybrid, data-parallel over batch (core c gets
batches [16c, 16c+16)):

- s in [0, 384)  (layout A, 12 MB fp16): SBUF partitions pack
  p = b*8 + g, group g owns s in [48g, 48g+48).  qb[128, H] fp16 is
  q[b] broadcast to every group row.  Each streamed chunk's s-columns
  run as fused multiply+reduce STTs on DVE (48 cols ~= 25.5 us).
  Batch-major packing makes the scores->softmax rearrange a single DMA:
  scores[(b g), c] -> scoresT[b, (g c)] have matching flat orders.
- s in [384, 512) (layout B, 4 MB fp16): enc packed [p=h%128,
  (b*8+hc)*128 + sc].  PE computes scp[1,128] = sum_hc qpe[:,hc*16+b]^T
  @ encB[:, (b,hc) block], PSUM-accumulated over the 8 h-chunks, for
  each of the 16 batches; ACT copies each scp row into
  scoresT[b, 384:512].  qpe[128, 8*16] (qT layout: [h-in-chunk,
  hc*16+b]) comes from 64 small PE matmuls W_chunk^T @ hidT.
- Both HWDGE rings carry the streams: A chunks alternate sync/scalar,
  B chunks alternate scalar/sync, W halves split across rings.
- Softmax on scoresT[16, 512]: reduce_max(negate) + exp(bias=-max,
  accum sum) + reciprocal + scale.

Host-permuted fp16 layouts (per core c, b0 = 16c):
    encA[b*8+g, c*H+h]        = fp16(enc[48g+c,  b0+b, h])      c<48
    encB[p, (b*8+hc)*128+sc]  = fp16(enc[384+sc, b0+b, hc*128+p])
    wh[p, half*KC*512+kc*512+j] = fp16(W[kc*128+p, half*512+j])
    hid[p, kc*128 + b*8+g]    = fp16(hidden[0, b0+b, kc*128+p])
    hidt[p, kc*16 + b]        = fp16(hidden[0, b0+b, kc*128+p])
"""

import os
import sys

for _p in ("/opt/trn_rl_repo", "/root/.axon_site/_ro/trn_rl_repo"):
    if os.path.isdir(_p):
        sys.path.insert(0, _p)
        break

from contextlib import ExitStack

import numpy as np

import concourse.tile as tile
from concourse import bacc, mybir
from concourse.bass_utils import run_bass_kernel_spmd

S, B, H = 512, 128, 1024
NCORES = 8
BLOC = B // NCORES          # 16 batches per core
GROUPS = 8                  # partition groups for layout A
S_PE = 128                  # s-positions scored on PE (s in [384, 512))
S_A = S - S_PE              # s-positions scored on DVE
SGA = S_A // GROUPS         # 48 s-values per group
KC = H // 128               # 8 contraction chunks
SPLIT = [8, 8, 8, 8, 8, 4, 4]   # layout-A chunk widths (s-cols)
assert sum(SPLIT) == SGA
NB_PER_BCHUNK = 2           # batches per layout-B chunk
NBCHUNK = BLOC // NB_PER_BCHUNK

FP32 = mybir.dt.float32
FP16 = mybir.dt.float16
MUL = mybir.AluOpType.mult

_cache = {}
LAST_RESULTS = None


def _build_nc(reps=1):
    """Build the kernel module.  reps>1 unrolls the kernel body
    back-to-back inside one program — used by the benchmark to measure
    marginal per-iteration device time.  kernel() always uses reps=1."""
    key = ("nc", reps)
    if key in _cache:
        return _cache[key]

    nc = bacc.Bacc(
        "TRN2",
        target_bir_lowering=False,
        debug=False,
        enable_asserts=True,
        num_devices=NCORES,
    )
    enc_d = nc.dram_tensor("enc", [128, SGA * H], FP16, kind="ExternalInput").ap()
    encb_d = nc.dram_tensor(
        "encb", [128, BLOC * KC * 128], FP16, kind="ExternalInput"
    ).ap()
    wh_d = nc.dram_tensor("wh", [128, KC * H], FP16, kind="ExternalInput").ap()
    hid_d = nc.dram_tensor("hid", [128, KC * 128], FP16, kind="ExternalInput").ap()
    hidt_d = nc.dram_tensor(
        "hidt", [128, KC * BLOC], FP16, kind="ExternalInput"
    ).ap()
    out = nc.dram_tensor("attn", [BLOC, S], FP32, kind="ExternalOutput").ap()

    with tile.TileContext(nc) as tc, ExitStack() as ctx:
        const_pool = ctx.enter_context(tc.tile_pool(name="const", bufs=1))
        w_pool = ctx.enter_context(tc.tile_pool(name="w", bufs=1))
        enc_pool = ctx.enter_context(tc.tile_pool(name="enc", bufs=6))
        scratch_pool = ctx.enter_context(tc.tile_pool(name="scratch", bufs=2))
        small_pool = ctx.enter_context(tc.tile_pool(name="small", bufs=1))
        psum_pool = ctx.enter_context(tc.tile_pool(name="psum", bufs=2, space="PSUM"))

        # PE clock-gate warmup + Exp activation table preload.
        wu = const_pool.tile([128, 512], FP16)
        nc.gpsimd.memset(wu[:], 1.0)
        wp = psum_pool.tile([1, 512], FP32, tag="wu")
        for _ in range(10):
            nc.tensor.matmul(wp[:], wu[:, 0:1], wu[:], start=True, stop=True)
        actwarm = const_pool.tile([16, 1], FP32)
        nc.scalar.activation(
            actwarm[:], wu[0:16, 0:1], mybir.ActivationFunctionType.Exp
        )

        for _rep in range(reps):
            _kernel_body(nc, tc, ctx, enc_d, encb_d, wh_d, hid_d, hidt_d, out,
                         const_pool, w_pool, enc_pool, scratch_pool,
                         small_pool, psum_pool)

    nc.finalize()
    _cache[key] = nc
    return nc


def _kernel_body(nc, tc, ctx, enc_d, encb_d, wh_d, hid_d, hidt_d, out,
                 const_pool, w_pool, enc_pool, scratch_pool, small_pool,
                 psum_pool):
    hid_sb = w_pool.tile([128, KC * 128], FP16, tag="hid_sb")
    hidt_sb = w_pool.tile([128, KC * BLOC], FP16, tag="hidt_sb")
    wh0_sb = w_pool.tile([128, KC * 512], FP16, tag="wh0_sb")
    wh1_sb = w_pool.tile([128, KC * 512], FP16, tag="wh1_sb")
    # hid + hidt + W half1 on the ACT ring; W half0 leads the SP ring.
    nc.scalar.dma_start(hid_sb[:], hid_d)
    nc.scalar.dma_start(hidt_sb[:], hidt_d)
    half_w = KC * 512
    nc.sync.dma_start(wh0_sb[:], wh_d[:, :half_w])
    nc.scalar.dma_start(wh1_sb[:], wh_d[:, half_w:])

    # ---- qb = broadcast(hidden @ W) for the DVE path ----
    qb = const_pool.tile([128, H], FP16, tag="qb")
    for half, wh_sb in enumerate((wh0_sb, wh1_sb)):
        qp = psum_pool.tile([128, 512], FP32, tag="qp")
        for kc in range(KC):
            nc.tensor.matmul(
                qp[:],
                hid_sb[:, kc * 128 : (kc + 1) * 128],
                wh_sb[:, kc * 512 : (kc + 1) * 512],
                start=(kc == 0),
                stop=(kc == KC - 1),
            )
        nc.scalar.copy(qb[:, half * 512 : (half + 1) * 512], qp[:])

    # ---- qpe = qT in [h-in-chunk, hc*16+b] layout for the PE path ----
    # qT[hc*128+h', b] = sum_k W[k, hc*128+h'] * hidden[b, k], done as 8
    # accumulation chains of W_chunk[128k, 128h]^T @ hidT[128k, 16b].
    qpe = const_pool.tile([128, KC * BLOC], FP16, tag="qpe")
    for hc in range(KC):
        half, j0 = hc // 4, (hc % 4) * 128
        wh_sb = wh0_sb if half == 0 else wh1_sb
        qtp = psum_pool.tile([128, BLOC], FP32, tag="qtp")
        for kc in range(KC):
            nc.tensor.matmul(
                qtp[:],
                wh_sb[:, kc * 512 + j0 : kc * 512 + j0 + 128],
                hidt_sb[:, kc * BLOC : (kc + 1) * BLOC],
                start=(kc == 0),
                stop=(kc == KC - 1),
            )
        nc.scalar.copy(qpe[:, hc * BLOC : (hc + 1) * BLOC], qtp[:])

    # ---- stream layout A (DVE fused STT) + layout B (PE) ----
    scores = small_pool.tile([128, SGA], FP32, tag="scores")
    scoresT = small_pool.tile([BLOC, S], FP32, tag="scoresT")
    a_ring = (nc.sync, nc.scalar)
    b_ring = (nc.scalar, nc.sync)

    def a_chunk(ch, col0, ncols):
        et = enc_pool.tile([128, ncols * H], FP16, tag="enc")
        a_ring[ch % 2].dma_start(et[:], enc_d[:, col0 * H : (col0 + ncols) * H])
        for j in range(ncols):
            prod = scratch_pool.tile([128, H], FP16, tag=f"prod{j}")
            nc.vector.scalar_tensor_tensor(
                out=prod[:], in0=et[:, j * H : (j + 1) * H], scalar=1.0,
                in1=qb[:], op0=MUL, op1=MUL,
                accum_out=scores[:, col0 + j : col0 + j + 1],
            )

    # PE scores land as columns: scp[sc, 0] = scores[b, 384+sc], collected
    # into pescores[128 sc, 16 b] (padded to 32 cols for the DVE 32x32
    # block transpose), then one transpose + one lane-aligned copy moves
    # them into scoresT[:, 384:].  Engines cannot shift partitions, so the
    # b-row never appears as a partition offset on a compute op.
    pescores = small_pool.tile([S_PE, 32], FP32, tag="pescores")
    nc.gpsimd.memset(pescores[:], 0.0)

    def b_chunk(k):
        ebt = enc_pool.tile([128, NB_PER_BCHUNK * KC * 128], FP16, tag="encb")
        off = k * NB_PER_BCHUNK * KC * 128
        b_ring[k % 2].dma_start(
            ebt[:], encb_d[:, off : off + NB_PER_BCHUNK * KC * 128]
        )
        for bi in range(NB_PER_BCHUNK):
            b = k * NB_PER_BCHUNK + bi
            scp = psum_pool.tile([S_PE, 1], FP32, tag="scp")
            for hc in range(KC):
                blk = (bi * KC + hc) * 128
                nc.tensor.matmul(
                    scp[:],
                    ebt[:, blk : blk + 128],
                    qpe[:, hc * BLOC + b : hc * BLOC + b + 1],
                    start=(hc == 0),
                    stop=(hc == KC - 1),
                )
            nc.scalar.copy(pescores[:, b : b + 1], scp[:])

    col0 = 0
    for ch, ncols in enumerate(SPLIT):
        a_chunk(ch, col0, ncols)
        col0 += ncols
        if ch < NBCHUNK:
            b_chunk(ch)
    for k in range(len(SPLIT), NBCHUNK):
        b_chunk(k)

    # PE columns -> scoresT rows.  vector.transpose flips each 32x32
    # block in place: pet[32i+b, sc'] = pescores[32i+sc', b].  Batch b of
    # s-block i then sits on partition 32i+b; DMAs (which, unlike compute
    # engines, can shift partitions) drop the four blocks into place.
    pet = small_pool.tile([S_PE, 32], FP32, tag="pet")
    nc.vector.transpose(pet[:], pescores[:])
    for i in range(S_PE // 32):
        nc.sync.dma_start(
            scoresT[:, S_A + 32 * i : S_A + 32 * (i + 1)],
            pet[32 * i : 32 * i + BLOC, :],
        )

    # scores[(b g), c] -> scoresT[b, (g c)] in one DMA (flat orders match).
    scoresT3 = scoresT[:, :S_A].rearrange("b (g c) -> b g c", g=GROUPS)
    nc.sync.dma_start(scoresT3, scores[:])

    # ---- softmax over s per batch ----
    mx = small_pool.tile([BLOC, 1], FP32, tag="mx")
    nc.vector.reduce_max(mx[:], scoresT[:], axis=mybir.AxisListType.X, negate=True)
    probs = small_pool.tile([BLOC, S], FP32, tag="probs")
    ssum = small_pool.tile([BLOC, 1], FP32, tag="ssum")
    nc.scalar.activation(
        probs[:],
        scoresT[:],
        mybir.ActivationFunctionType.Exp,
        bias=mx[:],
        scale=1.0,
        accum_out=ssum[:],
    )
    rsum = small_pool.tile([BLOC, 1], FP32, tag="rsum")
    nc.vector.reciprocal(rsum[:], ssum[:])
    attn_sb = small_pool.tile([BLOC, S], FP32, tag="attn_sb")
    nc.vector.tensor_scalar_mul(attn_sb[:], probs[:], rsum[:])
    nc.sync.dma_start(out, attn_sb[:])


def _prep_core_inputs(hid16_full, enc, w16, c):
    b0 = c * BLOC
    hidt = np.ascontiguousarray(
        hid16_full[:, :, b0 : b0 + BLOC].reshape(128, KC * BLOC)
    )
    hid16 = np.ascontiguousarray(
        np.repeat(hid16_full[:, :, b0 : b0 + BLOC], GROUPS, axis=2).reshape(
            128, KC * 128
        )
    )
    el = enc[:, b0 : b0 + BLOC, :]  # [512, 16, 1024] fp32
    encA = np.ascontiguousarray(
        el[:S_A]
        .reshape(GROUPS, SGA, BLOC, H)
        .transpose(2, 0, 1, 3)  # [b, g, c, h] -> partitions p = b*8+g
        .reshape(128, SGA * H)
        .astype(np.float16)
    )
    encB = np.ascontiguousarray(
        el[S_A:]
        .reshape(S_PE, BLOC, KC, 128)
        .transpose(3, 1, 2, 0)  # [p, b, hc, sc]
        .reshape(128, BLOC * KC * 128)
        .astype(np.float16)
    )
    return {"enc": encA, "encb": encB, "wh": w16, "hid": hid16, "hidt": hidt}


def _prep_in_maps(inputs):
    hidden = np.asarray(inputs["hidden"], dtype=np.float32)
    enc = np.asarray(inputs["encoder_outputs"], dtype=np.float32)
    w = np.asarray(inputs["W_attn"], dtype=np.float32)
    # wh16[p, half*KC*512 + kc*512 + j] = W[kc*128+p, half*512+j]
    wr = w.reshape(KC, 128, 2, 512).transpose(1, 2, 0, 3).reshape(128, KC * H)
    w16 = np.ascontiguousarray(wr.astype(np.float16))
    # hid16_full[p, kc, b] = hidden[0, b, kc*128+p]
    hid16_full = np.ascontiguousarray(
        hidden[0].reshape(B, KC, 128).transpose(2, 1, 0).astype(np.float16)
    )
    return [_prep_core_inputs(hid16_full, enc, w16, c) for c in range(NCORES)]


def _warmup():
    """Compile + run once on dummy inputs at import time so the first real
    kernel() call hits the in-process caches."""
    if _cache.get("warm") or os.environ.get("KERNEL_SKIP_WARMUP"):
        return
    try:
        kernel(
            np.zeros((1, B, H), np.float32),
            np.zeros((S, B, H), np.float32),
            np.zeros((H, H), np.float32),
            np.zeros((H,), np.float32),
        )
        _cache["warm"] = True
    except Exception:
        pass


def kernel(hidden, encoder_outputs, W_attn, b_attn=None, **_unused):
    global LAST_RESULTS
    nc = _build_nc()
    in_maps = _prep_in_maps(
        {"hidden": hidden, "encoder_outputs": encoder_outputs, "W_attn": W_attn}
    )
    res = run_bass_kernel_spmd(nc, in_maps, core_ids=list(range(NCORES)))
    LAST_RESULTS = res
    attn = np.concatenate([res.results[c]["attn"] for c in range(NCORES)], axis=0)
    return attn[:, None, :].astype(np.float32)


_warmup()
